# revision 1
# baseline (speedup 1.0000x reference)
"""Trainium2 Bass kernel for PrivateGraphSAGE (2-layer PrivSAGEConv).

Math per layer (reference):
    xc  = x / max(||x||_2 / 1.0, 1)          # per-row L2 clip
    msg = segment_sum(xc[src], dst, N)
    agg = xc + msg + noise
    out = agg @ W.T + b                       # b == 0 in this problem
Layer 1 is followed by SELU; layer 2 is the raw output.

Distribution strategy (8 NeuronCores, SPMD):
  - Nodes (x, noise, output) are sharded across cores (6250 rows each).
  - Each core computes the clipped+scaled table for its shard, then an
    AllGather materializes the full scaled table on every core.
  - Edges are partitioned by destination shard, then bucketed by
    (512-dst chunk, src half, 128-dst subchunk) and padded to groups of
    128 edges.  Group counts are maxed across cores so all cores run the
    identical program (pad edges gather row 0 and carry a -1 dst tag so
    they contribute nothing).
  - Per 128-edge group: dma_gather pulls the 128 source rows (512B each),
    a one-hot (built on DVE by comparing dst tags against an iota row)
    is used as the stationary matmul operand so the TensorEngine performs
    the segment-sum scatter into PSUM.
  - Self term + DP noise are added, a PE transpose + matmul with W.T
    finishes the layer; SELU and the next layer's clip are fused into the
    epilogue.  The inter-layer table is AllGathered once.
"""

import math

import numpy as np

import concourse.bacc as bacc
import concourse.bass as bass
import concourse.mybir as mybir
import concourse.tile as tile
from concourse.bass_utils import run_bass_kernel_spmd

F32 = mybir.dt.float32
BF16 = mybir.dt.bfloat16   # storage dtype of the gathered node tables
I16 = mybir.dt.int16

SUB = 128     # dst rows covered by one PSUM scatter target
CHUNK = 512   # dst rows per gather macro-chunk
GRP = 128     # edges per matmul group

SELU_LAM = 1.0507009873554804934193349852946
SELU_ALPHA = 1.6732632423543772848170429916717


# ---------------------------------------------------------------------------
# Host-side preprocessing
# ---------------------------------------------------------------------------

def _preprocess(src, dst, n_nodes, ncores):
    """Bucket edges (plus self edges) by (core, chunk, half, sub) and pad
    each bucket to a multiple of 128 edges using a group count that is
    uniform across cores.

    Returns meta dict (compile-time tables, identical for all cores) and
    per-core arrays (int16 gather indices, f32 dst tags)."""
    S = -(-n_nodes // ncores)            # shard rows per core
    nch = -(-S // CHUNK)                 # chunks per core
    s_pad = nch * CHUNK
    ntab = ncores * S                    # gather table rows (>= n_nodes)
    # split gather table into two halves so indices fit in int16
    H = (ntab // 2 + 127) // 128 * 128
    assert H <= 32768 and (ntab - H) <= 32768, (H, ntab)

    nodes = np.arange(n_nodes, dtype=np.int64)
    s_all = np.concatenate([np.asarray(src, np.int64), nodes])
    d_all = np.concatenate([np.asarray(dst, np.int64), nodes])

    core = np.minimum(d_all // S, ncores - 1)
    dloc = d_all - core * S
    chunk = dloc // CHUNK
    subq = (dloc % CHUNK) // SUB
    rel = dloc % SUB
    half = (s_all >= H).astype(np.int64)
    ihalf = s_all - half * H

    nb_per_core = nch * 2 * 4
    key = ((core * nch + chunk) * 2 + half) * 4 + subq
    order = np.argsort(key, kind="stable")
    key_s = key[order]
    ihalf_s = ihalf[order]
    rel_s = rel[order]

    counts = np.bincount(key_s, minlength=ncores * nb_per_core)
    G_percore = -(-counts // GRP)
    G = G_percore.reshape(ncores, nch, 2, 4).max(axis=0)   # [nch, 2, 4]

    # padded layout (chunk-major, then half, then sub), same for all cores
    bucket_len = (G * GRP).reshape(-1)                     # [nb_per_core]
    bucket_start = np.concatenate([[0], np.cumsum(bucket_len)[:-1]])
    e_pad = int(bucket_len.sum())
    g_tot = e_pad // GRP

    # per-edge destination offset inside its core's padded array
    run_start = np.concatenate([[0], np.cumsum(counts)[:-1]])
    within = np.arange(len(key_s)) - run_start[key_s]
    local_bucket = key_s % nb_per_core
    dest = bucket_start[local_bucket] + within

    idx_pad = np.zeros((ncores, e_pad), np.int64)
    rel_pad = np.full((ncores, e_pad), -1.0, np.float32)
    core_s = key_s // nb_per_core
    idx_pad[core_s, dest] = ihalf_s
    rel_pad[core_s, dest] = rel_s

    # ---- int16 gather-index tensor, [128, F_total] per core -------------
    # per (chunk, half) region, index j lives at [j % 16, col0 + j // 16];
    # the 16-row wrapped pattern is replicated across all eight 16-row
    # bands because different Q7 ucode versions read different bands
    # (the deployed one reads partitions 16..31).
    seg_len = (G * GRP).sum(axis=2).reshape(-1)            # [(nch*2)]
    seg_start = np.concatenate([[0], np.cumsum(seg_len)[:-1]])
    f_total = e_pad // 16
    idx16 = np.full((ncores, 128, f_total), 0, np.int16)
    for r in range(nch * 2):
        L = int(seg_len[r])
        if L == 0:
            continue
        s0 = int(seg_start[r])
        c0 = s0 // 16
        seg = idx_pad[:, s0:s0 + L]                        # [ncores, L]
        wrapped = seg.reshape(ncores, L // 16, 16).transpose(0, 2, 1)
        idx16[:, :, c0:c0 + L // 16] = np.tile(wrapped, (1, 8, 1)).astype(np.int16)

    # ---- f32 dst-tag tensor, [128, g_tot] per core ----------------------
    dstrel = rel_pad.reshape(ncores, g_tot, GRP).transpose(0, 2, 1).copy()

    meta = dict(
        ncores=ncores, n_nodes=n_nodes, S=S, nch=nch, s_pad=s_pad,
        ntab=ntab, H=H, e_pad=e_pad, g_tot=g_tot, f_total=f_total,
        G=G,                       # [nch, 2, 4] group counts
        seg_start=seg_start,       # flat (chunk, half) edge offsets
        seg_len=seg_len,
    )
    return meta, idx16, dstrel


# ---------------------------------------------------------------------------
# Device program
# ---------------------------------------------------------------------------

def _build_program(meta, with_b):
    m = meta
    nch, G = m["nch"], m["G"]
    ncores, S, s_pad, ntab, H = m["ncores"], m["S"], m["s_pad"], m["ntab"], m["H"]
    rg = [list(range(ncores))]

    nc = bacc.Bacc(None, target_bir_lowering=False)

    xs = nc.declare_dram_parameter("xs", [s_pad, 128], F32, isOutput=False)
    n1s = nc.declare_dram_parameter("n1s", [s_pad, 128], F32, isOutput=False)
    n2s = nc.declare_dram_parameter("n2s", [s_pad, 128], F32, isOutput=False)
    w1t = nc.declare_dram_parameter("w1t", [128, 128], F32, isOutput=False)
    w2t = nc.declare_dram_parameter("w2t", [128, 128], F32, isOutput=False)
    idxp = nc.declare_dram_parameter("idx", [128, m["f_total"]], I16, isOutput=False)
    drel = nc.declare_dram_parameter("dstrel", [128, m["g_tot"]], F32, isOutput=False)
    iotap = nc.declare_dram_parameter("iota", [128, 128], F32, isOutput=False)
    identp = nc.declare_dram_parameter("ident", [128, 128], F32, isOutput=False)
    if with_b:
        b1p = nc.declare_dram_parameter("b1r", [1, 128], F32, isOutput=False)
        b2p = nc.declare_dram_parameter("b2r", [1, 128], F32, isOutput=False)
    outp = nc.declare_dram_parameter("out", [s_pad, 128], F32, isOutput=True)

    xcs = nc.dram_tensor("xc_shard", [s_pad, 128], BF16)
    hcs = nc.dram_tensor("hc_shard", [s_pad, 128], BF16)
    xcf = nc.dram_tensor("xc_full", [ntab, 128], BF16, addr_space="Shared")
    hcf = nc.dram_tensor("hc_full", [ntab, 128], BF16, addr_space="Shared")

    mult = mybir.AluOpType.mult
    add = mybir.AluOpType.add
    is_eq = mybir.AluOpType.is_equal
    bypass = mybir.AluOpType.bypass
    Act = mybir.ActivationFunctionType

    from concourse.library_config import mlp
    nc.gpsimd.load_library(mlp)

    with tile.TileContext(nc) as tc:
        import contextlib
        with contextlib.ExitStack() as ctx:
            cpool = ctx.enter_context(tc.tile_pool(name="const", bufs=1))
            pa = ctx.enter_context(tc.tile_pool(name="pa", bufs=4))
            pa1 = ctx.enter_context(tc.tile_pool(name="pa1", bufs=4))
            gp = ctx.enter_context(tc.tile_pool(name="gather", bufs=2))
            ohp = ctx.enter_context(tc.tile_pool(name="onehot", bufs=4))
            ep = ctx.enter_context(tc.tile_pool(name="epil", bufs=4))
            eps = ctx.enter_context(tc.tile_pool(name="epilsc", bufs=4))
            psA = ctx.enter_context(tc.tile_pool(name="psA", bufs=3, space="PSUM"))
            psT = ctx.enter_context(tc.tile_pool(name="psT", bufs=2, space="PSUM"))
            psO = ctx.enter_context(tc.tile_pool(name="psO", bufs=2, space="PSUM"))

            # ---- constants -------------------------------------------------
            w1t_sb = cpool.tile([128, 128], F32, tag="w1t")
            nc.sync.dma_start(w1t_sb[:], w1t[:])
            w2t_sb = cpool.tile([128, 128], F32, tag="w2t")
            nc.sync.dma_start(w2t_sb[:], w2t[:])
            iota_sb = cpool.tile([128, 128], F32, tag="iota")
            nc.sync.dma_start(iota_sb[:], iotap[:])
            ident_sb = cpool.tile([128, 128], F32, tag="ident")
            nc.sync.dma_start(ident_sb[:], identp[:])
            idx_sb = cpool.tile([128, m["f_total"]], I16, tag="idx")
            nc.sync.dma_start(idx_sb[:], idxp[:])
            drel_sb = cpool.tile([128, m["g_tot"]], F32, tag="drel")
            nc.sync.dma_start(drel_sb[:], drel[:])
            if with_b:
                b1_sb = cpool.tile([1, 128], F32, tag="b1")
                nc.sync.dma_start(b1_sb[:], b1p[:])
                b2_sb = cpool.tile([1, 128], F32, tag="b2")
                nc.sync.dma_start(b2_sb[:], b2p[:])
                ones_sb = cpool.tile([1, 128], F32, tag="ones")
                nc.gpsimd.memset(ones_sb[:], 1.0)
            lnal_sb = cpool.tile([128, 1], F32, tag="lnal")
            nc.gpsimd.memset(lnal_sb[:], float(np.log(SELU_ALPHA)))

            # ---- phase A: clip+scale own shard of x ------------------------
            for t in range(s_pad // 128):
                rows = slice(t * 128, (t + 1) * 128)
                xt = pa.tile([128, 128], F32, tag="xt")
                nc.sync.dma_start(xt[:], xs[rows, :])
                sq = pa.tile([128, 128], F32, tag="sq")
                ss = pa1.tile([128, 1], F32, tag="ss")
                nc.scalar.activation(sq[:], xt[:], Act.Square, accum_out=ss[:])
                nrm = pa1.tile([128, 1], F32, tag="nrm")
                nc.scalar.activation(nrm[:], ss[:], Act.Sqrt)
                dd = pa1.tile([128, 1], F32, tag="dd")
                nc.vector.tensor_scalar_max(dd[:], nrm[:], 1.0)
                sc = pa1.tile([128, 1], F32, tag="sc")
                nc.vector.reciprocal(sc[:], dd[:])
                xc = pa.tile([128, 128], BF16, tag="xc")
                nc.vector.tensor_tensor(xc[:], xt[:], sc[:].to_broadcast([128, 128]), op=mult)
                nc.sync.dma_start(xcs[rows, :], xc[:])

            nc.gpsimd.collective_compute(
                "AllGather", bypass, ins=[xcs[:S, :]], outs=[xcf[:, :]],
                replica_groups=rg)

            # ---- one layer -------------------------------------------------
            def layer(src_tab, noise, wt_sb, b_sb, dst_shard, selu):
                lo_tab = src_tab[0:H, :]
                hi_tab = src_tab[H:ntab, :]
                MAXG = 8    # ≤1024 idxs per dma_gather: 64 descs/engine is
                            # the single-packet cap on the deployed ucode
                for ch in range(nch):
                    gts = {}
                    for h in (0, 1):
                        ng = int(G[ch, h, :].sum())
                        L = ng * GRP
                        if L == 0:
                            continue
                        r = ch * 2 + h
                        c0 = int(m["seg_start"][r]) // 16
                        gt = gp.tile([128, L], BF16, tag=f"g{h}")
                        tab = lo_tab if h == 0 else hi_tab
                        for g0 in range(0, ng, MAXG):
                            gspan = min(MAXG, ng - g0)
                            Ls = gspan * GRP
                            nc.gpsimd.dma_gather(
                                gt[:, g0 * GRP:g0 * GRP + Ls].rearrange(
                                    "p (g e) -> p g e", e=128),
                                tab,
                                idx_sb[:, c0 + g0 * 8:c0 + g0 * 8 + Ls // 16],
                                Ls, Ls, 128)
                        gts[h] = gt
                    # dst-tag column offset of first group of this chunk
                    gcol = int(m["seg_start"][ch * 2]) // GRP
                    # per-(half, sub) group column ranges, chunk-local order
                    # is (half, sub) to match the gather tiles
                    for su in range(4):
                        n_grp = int(G[ch, :, su].sum())
                        if n_grp == 0:
                            continue
                        pag = psA.tile([128, 128], F32, tag="pag")
                        done = 0
                        for h in (0, 1):
                            gs = int(G[ch, h, su])
                            if gs == 0:
                                continue
                            # column offset of (ch, h, su) in dstrel
                            c = gcol
                            if h == 1:
                                c += int(G[ch, 0, :].sum())
                            c += int(G[ch, h, :su].sum())
                            # free offset inside the gather tile
                            goff = int(G[ch, h, :su].sum())
                            oh = ohp.tile([128, gs * 128], BF16, tag="oh")
                            _build_onehot(nc, oh, drel_sb, c, gs, iota_sb)
                            for g in range(gs):
                                nc.tensor.matmul(
                                    pag[:],
                                    lhsT=oh[:, g * 128:(g + 1) * 128],
                                    rhs=gts[h][:, (goff + g) * 128:(goff + g + 1) * 128],
                                    start=(done == 0), stop=(done == n_grp - 1))
                                done += 1
                        rows = slice(ch * CHUNK + su * SUB, ch * CHUNK + su * SUB + 128)
                        nz = ep.tile([128, 128], F32, tag="nz")
                        nc.sync.dma_start(nz[:], noise[rows, :])
                        agg = ep.tile([128, 128], F32, tag="agg")
                        nc.vector.tensor_tensor(agg[:], pag[:], nz[:], op=add)
                        pt = psT.tile([128, 128], F32, tag="pt")
                        nc.tensor.transpose(pt[:], agg[:], ident_sb[:])
                        agT = ep.tile([128, 128], F32, tag="agT")
                        nc.vector.tensor_copy(agT[:], pt[:])
                        po = psO.tile([128, 128], F32, tag="po")
                        if b_sb is not None:
                            nc.tensor.matmul(po[:], lhsT=ones_sb[:], rhs=b_sb[:],
                                             start=True, stop=False)
                            nc.tensor.matmul(po[:], lhsT=agT[:], rhs=wt_sb[:],
                                             start=False, stop=True)
                        else:
                            nc.tensor.matmul(po[:], lhsT=agT[:], rhs=wt_sb[:],
                                             start=True, stop=True)
                        if selu:
                            t0 = ep.tile([128, 128], F32, tag="t0")
                            nc.vector.tensor_scalar_min(t0[:], po[:], 0.0)
                            e_ = ep.tile([128, 128], F32, tag="e_")
                            nc.scalar.activation(e_[:], t0[:], Act.Exp,
                                                 bias=lnal_sb[:])
                            m_ = ep.tile([128, 128], F32, tag="m_")
                            nc.vector.tensor_scalar_max(m_[:], po[:], 0.0)
                            u_ = ep.tile([128, 128], F32, tag="u_")
                            nc.vector.tensor_tensor(u_[:], m_[:], e_[:], op=add)
                            hh = ep.tile([128, 128], F32, tag="hh")
                            nc.scalar.activation(hh[:], u_[:], Act.Copy,
                                                 bias=-SELU_LAM * SELU_ALPHA,
                                                 scale=SELU_LAM)
                            # fused clip of h for the next layer
                            sq2 = ep.tile([128, 128], F32, tag="sq2")
                            ss2 = eps.tile([128, 1], F32, tag="ss2")
                            nc.scalar.activation(sq2[:], hh[:], Act.Square,
                                                 accum_out=ss2[:])
                            nr2 = eps.tile([128, 1], F32, tag="nr2")
                            nc.scalar.activation(nr2[:], ss2[:], Act.Sqrt)
                            dd2 = eps.tile([128, 1], F32, tag="dd2")
                            nc.vector.tensor_scalar_max(dd2[:], nr2[:], 1.0)
                            sc2 = eps.tile([128, 1], F32, tag="sc2")
                            nc.vector.reciprocal(sc2[:], dd2[:])
                            hc = ep.tile([128, 128], BF16, tag="hc")
                            nc.vector.tensor_tensor(hc[:], hh[:],
                                                    sc2[:].to_broadcast([128, 128]),
                                                    op=mult)
                            nc.sync.dma_start(dst_shard[rows, :], hc[:])
                        else:
                            ob = ep.tile([128, 128], F32, tag="ob")
                            nc.vector.tensor_copy(ob[:], po[:])
                            nc.sync.dma_start(dst_shard[rows, :], ob[:])

            layer(xcf, n1s, w1t_sb, b1_sb if with_b else None, hcs, selu=True)
            nc.gpsimd.collective_compute(
                "AllGather", bypass, ins=[hcs[:S, :]], outs=[hcf[:, :]],
                replica_groups=rg)
            layer(hcf, n2s, w2t_sb, b2_sb if with_b else None, outp, selu=False)

    nc.compile()
    return nc


def _build_onehot(nc, oh, drel_sb, c, gs, iota_sb):
    """onehot[e, g*128 + d] = (dstrel[e, c+g] == d), built on DVE in one op."""
    d3 = drel_sb[:, c:c + gs].to_broadcast([128, gs, 128])
    ii = iota_sb[:]
    i3 = bass.AP(ii.tensor, ii.offset, [list(ii.ap[0]), [0, gs], list(ii.ap[1])])
    o3 = oh[:].rearrange("p (g e) -> p g e", e=128)
    nc.vector.tensor_tensor(o3, d3, i3, op=mybir.AluOpType.is_equal)


# ---------------------------------------------------------------------------
# Entry point
# ---------------------------------------------------------------------------

def _run(inputs, ncores=8, sim=False, trace=False):
    x = np.ascontiguousarray(np.asarray(inputs["x"], np.float32))
    ei = np.asarray(inputs["edge_index"], np.int64)
    w1 = np.asarray(inputs["W1"], np.float32)
    b1 = np.asarray(inputs["b1"], np.float32)
    w2 = np.asarray(inputs["W2"], np.float32)
    b2 = np.asarray(inputs["b2"], np.float32)
    no1 = np.asarray(inputs["noise1"], np.float32)
    no2 = np.asarray(inputs["noise2"], np.float32)

    n_nodes = x.shape[0]
    meta, idx16, dstrel = _preprocess(ei[0], ei[1], n_nodes, ncores)
    S, s_pad = meta["S"], meta["s_pad"]

    with_b = bool(np.any(b1) or np.any(b2))
    nc = _build_program(meta, with_b)

    def shard(arr, c):
        lo = c * S
        hi = min(lo + S, n_nodes)
        out = np.zeros((s_pad, 128), np.float32)
        out[:hi - lo] = arr[lo:hi]
        return out

    iota = np.tile(np.arange(128, dtype=np.float32), (128, 1))
    ident = np.eye(128, dtype=np.float32)
    in_maps = []
    for c in range(ncores):
        im = dict(
            xs=shard(x, c), n1s=shard(no1, c), n2s=shard(no2, c),
            w1t=np.ascontiguousarray(w1.T), w2t=np.ascontiguousarray(w2.T),
            idx=idx16[c], dstrel=dstrel[c], iota=iota, ident=ident,
        )
        if with_b:
            im["b1r"] = b1.reshape(1, 128).astype(np.float32)
            im["b2r"] = b2.reshape(1, 128).astype(np.float32)
        in_maps.append(im)

    if sim:
        from concourse.bass_interp import MultiCoreSim
        msim = MultiCoreSim(nc, ncores)
        for c in range(ncores):
            for k, v in in_maps[c].items():
                msim.cores[c].tensor(k)[:] = v
        msim.simulate()
        results = [{"out": np.array(msim.cores[c].tensor("out"))}
                   for c in range(ncores)]
        res = None
    else:
        res = run_bass_kernel_spmd(nc, in_maps, core_ids=list(range(ncores)),
                                   trace=trace)
        results = res.results

    parts = []
    for c in range(ncores):
        lo = c * S
        hi = min(lo + S, n_nodes)
        parts.append(results[c]["out"][:hi - lo])
    out = np.concatenate(parts, axis=0).astype(np.float32)
    return out, res


def kernel(**inputs) -> np.ndarray:
    out, _ = _run(inputs, ncores=8, sim=False)
    return out

